# revision 12
# baseline (speedup 1.0000x reference)
"""Trainium2 Bass kernel for nn_Downsample_v2 (Haar DWT subband sum).

Math: summing all four Haar subbands (LL+LH+HL+HH)/4 algebraically
collapses to out[b,c,i,j] = 0.5 * x[b,c,2i,2j] — a stride-2 spatial
downsample with a scale.

Data path (pure data-parallel over batch, 2 batches per core, 8 cores):
  - DMA in only the even rows of the shard (contiguous 2 KB bursts,
    row stride 4 KB) into SBUF tiles [128, K*512].
  - Vector engine: out[:, j] = 0.5 * in[:, 2j]  (stride-2 free-dim read).
  - DMA out contiguous [128, K*256] tiles.
  - Load/store DMAs alternate between the two HWDGE rings (SP/ACT).
Per-core HBM traffic: 64 MiB read + 32 MiB write — the floor given the
>=512B-burst constraint (odd rows are never read).

Fair-share pacing: one core alone sustains ~400 GB/s, but 8 cores
demand 8x400 = 3.2 TB/s from a ~2.87 TB/s chip HBM pool. The HW
resolves the oversubscription unfairly: a random subset of cores gets
throttled 15-20% (per-core spans 300-340 us) while the rest finish in
262 us; the score is the max across cores. So each core is paced to
~370 GB/s by appending dummy in-place vector ops (0.53 ns/elem +
~0.25 us/inst) after each tile's real multiply: they extend the vector
chain that releases the input-tile slot, throttling load-DMA issue to
one tile per ~17 us. That keeps most cores at a ~269 us floor while
deep buffering (BUFS=4) lets a core that transiently loses arbitration
catch back up. The last BUFS+1 tiles skip the dummies (they gate no
further loads, and pacing the final load would stall the vector tail).
Measured max-across-cores ~305 us vs 324-340 us unpaced.
"""

import numpy as np

import concourse.bacc as bacc
import concourse.mybir as mybir
from concourse.bass_utils import run_bass_kernel_spmd
from concourse.tile import TileContext

N_CORES = 8
B, C, H, W = 16, 64, 512, 512
BS = B // N_CORES            # batches per core
R_IN = BS * C * H            # input rows per core shard (of length W)
R_OUT = R_IN // 2            # output rows per core shard (of length W//2)
P = 128                      # SBUF partitions
K = 16                       # even rows packed per partition per tile
BUFS = 4
N_TILES = R_OUT // (P * K)
PACE_WIDTHS = (8192, 8192, 8192, 1500)  # ~14.8 us of dummy vector work per tile

_NC_CACHE = {}


def _build_nc():
    nc = bacc.Bacc("TRN2", target_bir_lowering=False, debug=False)
    xs = nc.dram_tensor("xs", [R_IN, W], mybir.dt.float32, kind="ExternalInput")
    ys = nc.dram_tensor("ys", [R_OUT, W // 2], mybir.dt.float32, kind="ExternalOutput")

    # Even input rows, tiled: [N_TILES, P, K, W]; partition p of tile t
    # holds even-rows t*P*K + p*K + k.
    xt = xs[0::2, :].rearrange("(t p k) w -> t p k w", p=P, k=K)
    # Matching contiguous output view: [N_TILES, P, K*(W//2)].
    yt = ys.rearrange("(t p k) w -> t p (k w)", p=P, k=K)

    with TileContext(nc) as tc:
        with tc.tile_pool(name="io", bufs=BUFS) as pool:
            for t in range(N_TILES):
                ld = nc.sync if t % 2 == 0 else nc.scalar
                st = nc.scalar if t % 2 == 0 else nc.sync
                tin = pool.tile([P, K * W], mybir.dt.float32, tag="in")
                ld.dma_start(
                    out=tin[:].rearrange("p (k w) -> p k w", k=K), in_=xt[t]
                )
                tout = pool.tile([P, K * (W // 2)], mybir.dt.float32, tag="out")
                nc.vector.tensor_scalar_mul(tout[:], tin[:, 0 : K * W : 2], 0.5)
                if t < N_TILES - BUFS - 1:
                    # Pacing: these gate the tin-slot release for load t+BUFS.
                    for wdt in PACE_WIDTHS:
                        nc.vector.tensor_scalar_mul(tin[:, 0:wdt], tin[:, 0:wdt], 1.0)
                st.dma_start(out=yt[t], in_=tout[:])
    nc.finalize()
    return nc


def kernel(**inputs) -> np.ndarray:
    x = np.asarray(inputs["x"], dtype=np.float32)
    assert x.shape == (B, C, H, W), x.shape

    if "nc" not in _NC_CACHE:
        _NC_CACHE["nc"] = _build_nc()
    nc = _NC_CACHE["nc"]

    in_maps = [
        {"xs": np.ascontiguousarray(x[c * BS : (c + 1) * BS]).reshape(R_IN, W)}
        for c in range(N_CORES)
    ]
    res = run_bass_kernel_spmd(nc, in_maps, core_ids=list(range(N_CORES)))
    out = np.concatenate(
        [r["ys"].reshape(BS, C, H // 2, W // 2) for r in res.results], axis=0
    )
    return out
